# revision 2
# baseline (speedup 1.0000x reference)
"""Trainium2 Bass kernel for nn_BasicLaplacePINN.

Computes out[b] = sigma[b] * Laplacian(u)(x[b]) for a 3->64->64->64->1 tanh MLP
over B=262144 collocation points, data-parallel across 8 NeuronCores.

Algorithm: forward-over-forward propagation of (value t, three Jacobian
directions Jz, Laplacian accumulator Lz) with d = 1 - t^2:
  Jz_{l+1} = (d_l . Jz_l) @ W_{l+1}
  Lap_h_l  = -2 t_l d_l sum_i Jz_l[i]^2  +  d_l Lap_z_l
  out      = sigma * (Lap_h_3 @ W4)

Mapping (per core: 32768 samples, 32 tiles of 1024):
  - Features on partitions: two 64-wide batch halves packed into 128 rows;
    batch on the free dim (512 columns per half). Weights are host-packed
    128x128 block-diagonal stationary operands so one matmul serves both
    halves; x is host-pre-transposed so every DMA is contiguous.
  - Sign-folded streams make each elementwise step ONE fused instruction:
      jh = (t^2-1).Jz   (sign-flipped J; squares are insensitive)
      a  = (t^2-1).t  = -t d
      uk[i] = a . 2Jz[i]^2,  v = (t^2-1).Lz~ = d Lap_z   (Lz~ = -Lap_z)
    and Lz~_{l+1} = (sum_i uk[i] + v) @ (-W_{l+1}) rides free in PSUM
    accumulation (no reduction ops). The first-layer seed folds the constant
    c1h2 = 2*sum_d W1[d,:]^2 into the host-packed Lz2 weight.
  - Engines: ACT does tanh/Square/jh1; DVE does the fused STT/TT ops and
    the sigma multiply; GPSIMD does nothing (measured ~4x slower than its
    cost model); PE runs 18 matmuls/tile.
  - Precision: value-path matmuls (z1,z2,z3) in fp32 (accuracy anchor);
    Jacobian+Laplacian matmuls in float32r (1 cyc/row vs fp32's 4).
    Measured on hardware: 4.9e-4 scale-relative absmax vs an fp64 oracle,
    311 us device time per full pass (8 cores in parallel).
"""

import sys

for _p in ("/opt/trn_rl_repo",):
    if _p not in sys.path:
        sys.path.insert(0, _p)

import math
import numpy as np

B, D, H = 262144, 3, 64
NCORES = 8
BC = B // NCORES          # samples per core
NB = 512                  # free-dim tile size (per batch half)
HALF = BC // 2

_CACHE = {}
LAP16 = False

# Matmul input dtype: float32r is bit-identical to float32 but the PE runs it
# single-pass (reduced mantissa, ~TF32): 4x faster at N>=256.
MM_F32R = False


def _build_nc(bc, nb, f32r=True, lap_r=True, lap16=False, merge_jv=False, fold_j1=False, main_bufs=2, zb=2, jb=1, lb=2, ob=1, reps=1, fuse_jh=True, use_gps=False, batch_in=1):
    import concourse.bass as bass
    import concourse.bacc as bacc
    import concourse.tile as tile
    from concourse import mybir

    f32 = mybir.dt.float32
    Tanh = mybir.ActivationFunctionType.Tanh
    Square = mybir.ActivationFunctionType.Square
    SUB = mybir.AluOpType.subtract
    MUL = mybir.AluOpType.mult
    AP = bass.AP

    half = bc // 2
    ntiles = half // nb
    assert ntiles * nb == half

    mdt = mybir.dt.float32r if f32r else f32
    ldt = mybir.dt.float16 if lap16 else (mdt if lap_r else f32)
    sdt = mybir.dt.float16 if lap16 else f32

    def mm(out, lhsT, rhs, **kw):
        nc.tensor.matmul(out, lhsT, rhs, **kw)

    nc = bacc.Bacc()
    # All host-prepacked:
    #   xt[3h+f, n] = x[h*half+n, f]                       (f32r view of f32 bits)
    #   wp2 = blockdiag(W2, W2), wn2 = -wp2, same for 3;   wp1 [6,128]; wp4 [128,2]
    #   bp* = per-feature bias replicated to 128 rows;     w1r [128,3]; c1h2 [128]
    xh = nc.dram_tensor("xt", [2 * D, bc // 2], f32, kind="ExternalInput")
    sgh = nc.dram_tensor("sg", [bc], f32, kind="ExternalInput")
    wp1h = nc.dram_tensor("wp1", [2 * D, 128], f32, kind="ExternalInput")
    wp2h = nc.dram_tensor("wp2", [128, 128], f32, kind="ExternalInput")
    wr2_shape = [D, 128, 128] if fold_j1 else [128, 128]
    wr2h = nc.dram_tensor("wr2", wr2_shape, mdt, kind="ExternalInput")
    wn2h = nc.dram_tensor("wn2", [128, 128], ldt, kind="ExternalInput")
    wp3h = nc.dram_tensor("wp3", [128, 128], f32, kind="ExternalInput")
    wr3h = nc.dram_tensor("wr3", [128, 128], mdt, kind="ExternalInput")
    wn3h = nc.dram_tensor("wn3", [128, 128], ldt, kind="ExternalInput")
    wp4h = nc.dram_tensor("wp4", [128, 2], ldt, kind="ExternalInput")
    bp1h = nc.dram_tensor("bp1", [128], f32, kind="ExternalInput")
    bp2h = nc.dram_tensor("bp2", [128], f32, kind="ExternalInput")
    bp3h = nc.dram_tensor("bp3", [128], f32, kind="ExternalInput")
    w1rh = nc.dram_tensor("w1r", [128, D], f32, kind="ExternalInput")
    w1rnh = nc.dram_tensor("w1rn", [128, D], f32, kind="ExternalInput")
    outh = nc.dram_tensor("out", [bc, 1], f32, kind="ExternalOutput")

    with tile.TileContext(nc) as tc:
        with (
            tc.tile_pool(name="consts", bufs=1) as consts,
            tc.tile_pool(name="main", bufs=main_bufs) as main,
            tc.tile_pool(name="ps", bufs=1, space="PSUM") as ps,
        ):
            # ---- constants: single-DMA loads of host-prepacked tensors ----
            w1p = consts.tile([2 * D, 128], f32, tag="w1p")
            nc.sync.dma_start(out=w1p, in_=wp1h[:, :])
            w2p = consts.tile([128, 128], f32, tag="w2p")
            nc.sync.dma_start(out=w2p, in_=wp2h[:, :])
            if fold_j1:
                w2r = consts.tile([128, D, 128], mdt, tag="w2r")
                nc.sync.dma_start(
                    out=w2r, in_=AP(wr2h, 0, [[128, 128], [16384, D], [1, 128]])
                )
            else:
                w2r = consts.tile([128, 128], mdt, tag="w2r")
                nc.sync.dma_start(out=w2r, in_=wr2h[:, :])
            w2nc = consts.tile([128, 128], ldt, tag="w2nc")
            nc.sync.dma_start(out=w2nc, in_=wn2h[:, :])
            w3p = consts.tile([128, 128], f32, tag="w3p")
            nc.sync.dma_start(out=w3p, in_=wp3h[:, :])
            w3r = consts.tile([128, 128], mdt, tag="w3r")
            nc.sync.dma_start(out=w3r, in_=wr3h[:, :])
            w3n = consts.tile([128, 128], ldt, tag="w3n")
            nc.sync.dma_start(out=w3n, in_=wn3h[:, :])
            w4p = consts.tile([128, 2], ldt, tag="w4p")
            nc.sync.dma_start(out=w4p, in_=wp4h[:, :])
            b1p = consts.tile([128, 1], f32, tag="b1p")
            nc.sync.dma_start(out=b1p, in_=AP(bp1h, 0, [[1, 128], [1, 1]]))
            b2p = consts.tile([128, 1], f32, tag="b2p")
            nc.sync.dma_start(out=b2p, in_=AP(bp2h, 0, [[1, 128], [1, 1]]))
            b3p = consts.tile([128, 1], f32, tag="b3p")
            nc.sync.dma_start(out=b3p, in_=AP(bp3h, 0, [[1, 128], [1, 1]]))
            w1rp = consts.tile([128, D], f32, tag="w1rp")
            nc.sync.dma_start(out=w1rp, in_=w1rh[:, :])
            w1rn = consts.tile([128, D], f32, tag="w1rn")
            nc.sync.dma_start(out=w1rn, in_=w1rnh[:, :])
            neg1 = consts.tile([128, 1], f32, tag="neg1")
            nc.vector.memset(neg1, -1.0)

            # ---- main loop over batch tiles (reps>1: timing-only repeat) ----
            rep_ctx = tc.For_i(0, reps, 1) if reps > 1 else None
            if rep_ctx is not None:
                rep_ctx.__enter__()
            xsb = sgsb = None
            for i in range(ntiles):
                if batch_in > 1:
                    g, k = divmod(i, batch_in)
                    if k == 0:
                        gn = min(batch_in, ntiles - i) * nb
                        xsb = main.tile([2 * D, batch_in * nb], f32, tag="xsb")
                        nc.sync.dma_start(
                            out=xsb[:, :gn],
                            in_=AP(xh, i * nb, [[half, 2 * D], [1, gn]]),
                        )
                        sgsb = main.tile([2, batch_in * nb], f32, tag="sgsb")
                        nc.sync.dma_start(
                            out=sgsb[:, :gn],
                            in_=AP(sgh, i * nb, [[half, 2], [1, gn]]),
                        )
                    xs = xsb[:, k * nb : (k + 1) * nb]
                    sgs = sgsb[:, k * nb : (k + 1) * nb]
                else:
                    xs = main.tile([2 * D, nb], f32, tag="xs")
                    nc.sync.dma_start(
                        out=xs, in_=AP(xh, i * nb, [[half, 2 * D], [1, nb]])
                    )
                    sgs = main.tile([2, nb], f32, tag="sgs")
                    nc.sync.dma_start(out=sgs, in_=AP(sgh, i * nb, [[half, 2], [1, nb]]))

                # ---- L1 ----
                zp1 = ps.tile([128, nb], f32, tag="z", bufs=zb)
                mm(zp1, w1p[:, :], xs[:, :], start=True, stop=True)
                t1 = main.tile([128, nb], f32, tag="t1", bufs=3)
                nc.scalar.activation(t1, zp1, Tanh, bias=b1p)
                tsq1 = main.tile([128, nb], f32, tag="tsq1", bufs=3)
                nc.scalar.activation(tsq1, t1, Square)
                if fold_j1:
                    dm1 = main.tile([128, nb], mdt, tag="jh1", bufs=3)
                    nc.scalar.activation(dm1, tsq1, mybir.ActivationFunctionType.Identity, bias=neg1)
                else:
                    jh1 = main.tile([128, D, nb], mdt, tag="jh1")
                    for d in range(D):
                        nc.scalar.activation(
                            jh1[:, d, :], tsq1, mybir.ActivationFunctionType.Identity,
                            bias=w1rn[:, d : d + 1], scale=w1rp[:, d : d + 1],
                        )
                a1 = main.tile([128, nb], ldt, tag="a1", bufs=3)
                nc.vector.scalar_tensor_tensor(a1, tsq1, 1.0, t1, SUB, MUL)

                # ---- L2 matmuls ----
                zp2 = ps.tile([128, nb], f32, tag="z", bufs=zb)
                mm(zp2, w2p, t1, start=True, stop=True)
                if merge_jv:
                    jp2 = ps.tile([128, D + 1, nb], f32, tag="j", bufs=jb)
                    lp2 = jp2[:, D, :]
                else:
                    jp2 = ps.tile([128, D, nb], f32, tag="j", bufs=jb)
                    lp2f = ps.tile([128, nb], f32, tag="l", bufs=lb)
                    lp2 = lp2f[:, :]
                for d in range(D):
                    if fold_j1:
                        mm(jp2[:, d, :], w2r[:, d, :], dm1, start=True, stop=True)
                    else:
                        mm(jp2[:, d, :], w2r, jh1[:, d, :], start=True, stop=True)
                mm(lp2, w2nc, a1, start=True, stop=True)

                # ---- L2 elementwise ----
                t2 = main.tile([128, nb], f32, tag="t2", bufs=3)
                nc.scalar.activation(t2, zp2, Tanh, bias=b2p)
                tsq2 = main.tile([128, nb], f32, tag="tsq2", bufs=3)
                nc.scalar.activation(tsq2, t2, Square)
                s2 = main.tile([128, D, nb], sdt, tag="s2")
                nc.scalar.activation(s2, jp2[:, 0:D, :], Square, scale=math.sqrt(2.0))
                nd2 = D + 1 if merge_jv else D
                jh2 = main.tile([128, nd2, nb], mdt, tag="jh2")
                tb2 = AP(tensor=tsq2.tensor, offset=tsq2.offset,
                         ap=[list(tsq2.ap[0]), [0, nd2]] + [list(p_) for p_ in tsq2.ap[1:]])
                nc.vector.scalar_tensor_tensor(jh2, tb2, 1.0, jp2[:, 0:nd2, :], SUB, MUL)
                a2 = main.tile([128, nb], sdt, tag="a2", bufs=3)
                nc.vector.scalar_tensor_tensor(a2, tsq2, 1.0, t2, SUB, MUL)
                uk2 = main.tile([128, D, nb], ldt, tag="uk2")
                ab2 = AP(tensor=a2.tensor, offset=a2.offset,
                         ap=[list(a2.ap[0]), [0, D]] + [list(p_) for p_ in a2.ap[1:]])
                nc.vector.tensor_mul(uk2, ab2, s2)
                if merge_jv:
                    v2 = jh2[:, D, :]
                else:
                    v2f = main.tile([128, nb], ldt, tag="v2", bufs=3)
                    nc.vector.scalar_tensor_tensor(v2f, tsq2, 1.0, lp2, SUB, MUL)
                    v2 = v2f[:, :]

                # ---- L3 matmuls ----
                zp3 = ps.tile([128, nb], f32, tag="z", bufs=zb)
                mm(zp3, w3p, t2, start=True, stop=True)
                if merge_jv:
                    jp3 = ps.tile([128, D + 1, nb], f32, tag="j", bufs=jb)
                    lp3 = jp3[:, D, :]
                else:
                    jp3 = ps.tile([128, D, nb], f32, tag="j", bufs=jb)
                    lp3f = ps.tile([128, nb], f32, tag="l", bufs=lb)
                    lp3 = lp3f[:, :]
                for d in range(D):
                    mm(jp3[:, d, :], w3r, jh2[:, d, :], start=True, stop=True)
                mm(lp3, w3n, uk2[:, 0, :], start=True, stop=False)
                mm(lp3, w3n, uk2[:, 1, :], start=False, stop=False)
                mm(lp3, w3n, uk2[:, 2, :], start=False, stop=False)
                mm(lp3, w3n, v2, start=False, stop=True)

                # ---- L3 elementwise ----
                t3 = main.tile([128, nb], f32, tag="t3", bufs=3)
                nc.scalar.activation(t3, zp3, Tanh, bias=b3p)
                tsq3 = main.tile([128, nb], f32, tag="tsq3", bufs=3)
                nc.scalar.activation(tsq3, t3, Square)
                s3 = main.tile([128, D, nb], sdt, tag="s3")
                nc.scalar.activation(s3, jp3[:, 0:D, :], Square, scale=math.sqrt(2.0))
                a3 = main.tile([128, nb], sdt, tag="a3", bufs=3)
                nc.vector.scalar_tensor_tensor(a3, tsq3, 1.0, t3, SUB, MUL)
                uk3 = main.tile([128, D, nb], ldt, tag="uk3")
                ab3 = AP(tensor=a3.tensor, offset=a3.offset,
                         ap=[list(a3.ap[0]), [0, D]] + [list(p_) for p_ in a3.ap[1:]])
                nc.vector.tensor_mul(uk3, ab3, s3)
                v3 = main.tile([128, nb], ldt, tag="v3", bufs=3)
                nc.vector.scalar_tensor_tensor(v3, tsq3, 1.0, lp3, SUB, MUL)

                # ---- L4 + output ----
                op4f = ps.tile([128, nb], f32, tag="o", bufs=ob)
                op4 = op4f[0:2, :]
                mm(op4, w4p, uk3[:, 0, :], start=True, stop=False)
                mm(op4, w4p, uk3[:, 1, :], start=False, stop=False)
                mm(op4, w4p, uk3[:, 2, :], start=False, stop=False)
                mm(op4, w4p, v3, start=False, stop=True)
                osb = main.tile([2, nb], f32, tag="osb")
                nc.vector.tensor_mul(osb, op4, sgs)
                nc.sync.dma_start(
                    out=AP(outh, i * nb, [[half, 2], [1, nb]]), in_=osb
                )
            if rep_ctx is not None:
                rep_ctx.__exit__(None, None, None)

    nc.compile()
    return nc


def _get_nc(bc=BC, nb=NB):
    key = (bc, nb)
    if key not in _CACHE:
        _CACHE[key] = _build_nc(bc, nb)
    return _CACHE[key]


def pack_consts(w1, b1, w2, b2, w3, b3, w4):
    """Host-side packing of block-diagonal weights and broadcast vectors."""
    f = np.float32

    def blockdiag(w):
        p = np.zeros((128, 128), f)
        p[:H, :H] = w
        p[H:, H:] = w
        return p

    wp1 = np.zeros((2 * D, 128), f)
    wp1[:D, :H] = w1
    wp1[D:, H:] = w1
    wp2, wp3 = blockdiag(w2), blockdiag(w3)
    wp4 = np.zeros((128, 2), np.float16 if LAP16 else f)
    wp4[:H, 0] = w4[:, 0]
    wp4[H:, 1] = w4[:, 0]
    c1h2 = 2.0 * (w1.astype(np.float64) ** 2).sum(0)
    lf = np.float16 if LAP16 else f
    wn2c = -(np.tile(c1h2, 2)[:, None].astype(np.float64) * blockdiag(w2)).astype(lf)
    return {
        "wp1": wp1, "wp2": wp2, "wn2": wn2c, "wp3": wp3, "wn3": (-wp3).astype(lf),
        "wr2": wp2, "wr3": wp3,
        "wp4": wp4,
        "bp1": np.tile(b1, 2).astype(f), "bp2": np.tile(b2, 2).astype(f),
        "bp3": np.tile(b3, 2).astype(f),
        "w1r": np.tile(w1.T, (2, 1)).astype(f),
        "w1rn": -np.tile(w1.T, (2, 1)).astype(f),
    }


def make_in_map(x, sg, consts, c):
    sl = slice(c * BC, (c + 1) * BC)
    xc = x[sl]
    xt = np.ascontiguousarray(
        np.concatenate([xc[:HALF].T, xc[HALF:].T], axis=0)
    )
    return {"xt": xt, "sg": sg[sl], **consts}


def assemble_out(res):
    out = np.concatenate([res.results[c]["out"] for c in range(NCORES)], axis=0)
    return out.astype(np.float32)


def kernel(**inputs):
    from concourse.bass_utils import run_bass_kernel_spmd

    f = lambda k: np.ascontiguousarray(np.asarray(inputs[k], dtype=np.float32))
    x, sg = f("x_r"), f("sigma_r")
    consts = pack_consts(
        f("W1"), f("b1"), f("W2"), f("b2"), f("W3"), f("b3"), f("W4")
    )

    nc = _get_nc()
    in_maps = [make_in_map(x, sg, consts, c) for c in range(NCORES)]
    res = run_bass_kernel_spmd(nc, in_maps, core_ids=list(range(NCORES)))
    return assemble_out(res)


if __name__ == "__main__":
    nc = _get_nc(2048, 512)
    print("built ok:", len(nc.m.functions[0].instructions) if hasattr(nc.m.functions[0], "instructions") else "n/a")

